# revision 1
# baseline (speedup 1.0000x reference)
"""Trainium2 Bass kernel for nn_CacheAttention (retrieval KNN attention).

Reference computation (per token, fully independent across tokens):
    q = (hidden @ Wq.T) * D**-0.5          # [t, H*D] viewed [t, KV, G, D]
    k = retrieved @ Wk.T                   # [t, N, KV*D] viewed [t, KV, N, D]
    v = retrieved @ Wv.T                   # viewed [t, KV, N, D]
    s = einsum('kgd,knd->kgn', q_t, k_t);  a = softmax(s, -1)
    out_t = einsum('kgn,knd->kgd', a, v_t).reshape(H*D) @ Wo.T

Strategy: data-parallel over the 4096 (b, s) tokens across 8 NeuronCores
(512 tokens each).  The host pre-transposes + bf16-casts all operands so
every matmul contracts over the SBUF partition dim with cheap, contiguous
DMA loads.  Attention uses a block-diagonal PE trick: scores for 32 tokens
x 4 groups in one PSUM tile [(g,t)=128, (t',n)=512], additive -30 mask +
exp (denominator via ACT accum), PE transpose of A, then a second
block-diag matmul against V for the weighted sum.
"""

import os
import sys

import numpy as np
import ml_dtypes

for _p in ("/opt/trn_rl_repo", "/root/.axon_site/_ro/trn_rl_repo"):
    if os.path.isdir(_p) and _p not in sys.path:
        sys.path.insert(0, _p)

import concourse.bass as bass  # noqa: E402
import concourse.mybir as mybir  # noqa: E402
import concourse.tile as tile  # noqa: E402
from concourse import bacc  # noqa: E402
from concourse.bass_utils import run_bass_kernel_spmd  # noqa: E402
from concourse.masks import make_identity  # noqa: E402

# Problem shapes (hardcoded per contest contract).
B, S, HID = 2, 2048, 4096
H, KV, D = 32, 8, 128
G = H // KV  # 4
N = 16
RH = HID // 4  # 1024
NCORES = 8
TOK = B * S  # 4096 tokens total
T = TOK // NCORES  # 512 tokens per core
TBLK = 32  # tokens per pipeline block
NBLK = T // TBLK  # 8
TN = T * N  # 8192 (token, neighbor) rows per core
TNBLK = TBLK * N  # 1024
SCALE = float(D) ** -0.5
MASK_NEG = -30.0

BF16 = mybir.dt.bfloat16
F32 = mybir.dt.float32
EXP = mybir.ActivationFunctionType.Exp

_NC = None


def _build_program(reps=1):
    nc = bacc.Bacc(None, target_bir_lowering=False, debug=False)

    hT = nc.dram_tensor("hT", [HID, T], BF16, kind="ExternalInput")
    rT = nc.dram_tensor("rT", [RH, TN], BF16, kind="ExternalInput")
    wqT = nc.dram_tensor("wqT", [HID, H * D], BF16, kind="ExternalInput")
    wkT = nc.dram_tensor("wkT", [RH, KV * D], BF16, kind="ExternalInput")
    wvT = nc.dram_tensor("wvT", [RH, KV * D], BF16, kind="ExternalInput")
    woT = nc.dram_tensor("woT", [H * D, HID], BF16, kind="ExternalInput")
    mneg = nc.dram_tensor("mneg", [128, 512], F32, kind="ExternalInput")
    out = nc.dram_tensor("out", [T, HID], F32, kind="ExternalOutput")

    hT_r = hT[:].rearrange("(c p) t -> p c t", p=128)
    rT_r = rT[:].rearrange("(c p) t -> p c t", p=128)
    wq_r = wqT[:].rearrange("(c p) m -> p c m", p=128)
    wk_r = wkT[:].rearrange("(c p) m -> p c m", p=128)
    wv_r = wvT[:].rearrange("(c p) m -> p c m", p=128)
    wo_r = woT[:].rearrange("(c p) h -> p c h", p=128)
    out_r = out[:].rearrange("(mt p) h -> p mt h", p=128)

    with tile.TileContext(nc) as tc:
      for _rep in range(reps):  # >1 only for timing calibration builds
        with tc.tile_pool(name="resident", bufs=1) as resp:
            mneg_sb = resp.tile([128, 512], F32)
            nc.sync.dma_start(mneg_sb[:], mneg[:])
            ident = resp.tile([128, 128], BF16)
            make_identity(nc, ident[:])
            wk_sb = resp.tile([128, RH // 128, KV * D], BF16)
            nc.sync.dma_start(wk_sb[:], wk_r)
            wv_sb = resp.tile([128, RH // 128, KV * D], BF16)
            nc.sync.dma_start(wv_sb[:], wv_r)
            # Filled by stage 1 / stage 2, consumed downstream.
            # Q^T layout [d, kv, (grp, g, t32)]: the scores stationary for
            # (kv, grp) is then a contiguous [128, 128] slice covering all
            # 4 groups g x 32 tokens (walrus rejects 3D weight APs).
            qT_sb = resp.tile([128, KV, G * T], BF16)
            qT_w = qT_sb[:].rearrange("p h (a g t) -> p h a g t", g=G, t=32)
            aoT_sb = resp.tile([128, H * D // 128, T], BF16)

            # ---- Stage 1: Q^T[(kv,g,d), t] = WqT.T-chunks x hT ----
            with (
                tc.tile_pool(name="hpool", bufs=1) as hp,
                tc.tile_pool(name="wq", bufs=48) as wqp,
                tc.tile_pool(name="ps1", bufs=2, space="PSUM") as ps1,
            ):
                hT_sb = hp.tile([128, HID // 128, T], BF16)
                nc.sync.dma_start(hT_sb[:], hT_r)
                for ms in range(8):  # 512-col slabs of Wq^T
                    slabs = []
                    for k in range(HID // 128):
                        sl = wqp.tile([128, 512], BF16, tag="wqslab")
                        nc.sync.dma_start(sl[:], wq_r[:, k, ms * 512 : (ms + 1) * 512])
                        slabs.append(sl)
                    for mi in range(4):
                        m = ms * 4 + mi
                        qps = ps1.tile([128, 512], F32, tag="qps")
                        for k in range(HID // 128):
                            nc.tensor.matmul(
                                qps[:],
                                slabs[k][:, mi * 128 : (mi + 1) * 128],
                                hT_sb[:, k, :],
                                start=(k == 0),
                                stop=(k == HID // 128 - 1),
                            )
                        # Fold the D**-0.5 query scaling into the PSUM evict.
                        nc.scalar.mul(
                            qT_w[:, m // G, :, m % G, :],
                            qps[:].rearrange("p (a t) -> p a t", t=32),
                            SCALE,
                        )

            # ---- Stage 2: per 64-token block: K/V projections + attention ----
            with (
                tc.tile_pool(name="rt", bufs=2) as rtp,
                tc.tile_pool(name="kt", bufs=2) as ktp,
                tc.tile_pool(name="vt", bufs=2) as vtp,
                tc.tile_pool(name="attn", bufs=2) as atp,
                tc.tile_pool(name="ps2", bufs=2, space="PSUM") as ps2,
                tc.tile_pool(name="pss", bufs=2, space="PSUM") as pss,
                tc.tile_pool(name="pstr", bufs=2, space="PSUM") as pstr,
                tc.tile_pool(name="psav", bufs=2, space="PSUM") as psav,
            ):
                for blk in range(NBLK):
                    t0 = blk * TBLK
                    rtile = rtp.tile([128, RH // 128, TNBLK], BF16, tag="rt")
                    nc.sync.dma_start(
                        rtile[:], rT_r[:, :, blk * TNBLK : (blk + 1) * TNBLK]
                    )

                    # The reference's torch flat-view [t,n,KV*D] -> [t,KV,n,D]
                    # means head kv attends slot nn drawn from neighbor
                    # n_src = 2*kv + nn//8 with kv-slice kvc = nn%8.  Softmax
                    # is permutation-invariant per head, so we only need a
                    # consistent slot order for K and V: slot = (n_src%2)*8
                    # + kvc, gathered on the free dim during PSUM eviction.
                    # K^T[d, head, (t, slot)] for this block.
                    ktile = ktp.tile([128, KV, TNBLK], BF16, tag="kt")
                    kdst = ktile[:].rearrange("p h (t b e) -> p t h b e", b=2, e=8)
                    for kvc in range(KV):
                        for f2 in range(TNBLK // 512):
                            kps = ps2.tile([128, 512], F32, tag="ps2")
                            for k in range(RH // 128):
                                nc.tensor.matmul(
                                    kps[:],
                                    wk_sb[:, k, kvc * 128 : (kvc + 1) * 128],
                                    rtile[:, k, f2 * 512 : f2 * 512 + 512],
                                    start=(k == 0),
                                    stop=(k == RH // 128 - 1),
                                )
                            nc.scalar.copy(
                                kdst[:, f2 * 32 : (f2 + 1) * 32, :, :, kvc],
                                kps[:].rearrange("p (t a b) -> p t a b", a=8, b=2),
                            )

                    # V^T[d, head, (t, slot)], same gather as K; then PE
                    # transpose per head-chunk to V_flat[(t,slot), d].
                    vht = vtp.tile([128, KV, TNBLK], BF16, tag="vht")
                    vdst = vht[:].rearrange("p h (t b e) -> p t h b e", b=2, e=8)
                    for kvc in range(KV):
                        for f2 in range(TNBLK // 512):
                            vps = ps2.tile([128, 512], F32, tag="ps2")
                            for k in range(RH // 128):
                                nc.tensor.matmul(
                                    vps[:],
                                    wv_sb[:, k, kvc * 128 : (kvc + 1) * 128],
                                    rtile[:, k, f2 * 512 : f2 * 512 + 512],
                                    start=(k == 0),
                                    stop=(k == RH // 128 - 1),
                                )
                            nc.scalar.copy(
                                vdst[:, f2 * 32 : (f2 + 1) * 32, :, :, kvc],
                                vps[:].rearrange("p (t a b) -> p t a b", a=8, b=2),
                            )
                    vflat = vtp.tile([128, KV, TNBLK // 128, D], BF16, tag="vflat")
                    for kv in range(KV):
                        for c in range(TNBLK // 128):
                            vtp_ps = pstr.tile([128, 128], BF16, tag="tps")
                            nc.tensor.transpose(
                                vtp_ps[:], vht[:, kv, c * 128 : (c + 1) * 128], ident[:]
                            )
                            nc.vector.tensor_copy(vflat[:, kv, c, :], vtp_ps[:])

                    # Attention for each (kv head, 32-token group).
                    for kv in range(KV):
                        for grp in range(TBLK // 32):
                            # Scores: psum [(g,t)=128, (t',n)=512]; only the
                            # block-diagonal (t'==t) 16-col slices are valid.
                            # The stationary packs all 4 GQA groups x 32
                            # tokens as one contiguous [128, 128] slice of
                            # the (grp, g, t) Q layout (walrus rejects >2D
                            # weight APs, so the layout bakes the packing).
                            sps = pss.tile([128, 512], F32, tag="sps")
                            gg = (t0 + grp * 32) // 32
                            nc.tensor.matmul(
                                sps[:],
                                qT_sb[:, kv, gg * 128 : (gg + 1) * 128],
                                ktile[:, kv, grp * 512 : grp * 512 + 512],
                                start=True,
                                stop=True,
                            )
                            sm = atp.tile([128, 512], F32, tag="sm")
                            nc.vector.tensor_add(sm[:], sps[:], mneg_sb[:])
                            e = atp.tile([128, 512], BF16, tag="e")
                            den = atp.tile([128, 1], F32, tag="den")
                            nc.scalar.activation(e[:], sm[:], EXP, accum_out=den[:])
                            rec = atp.tile([128, 1], F32, tag="rec")
                            nc.vector.reciprocal(rec[:], den[:])
                            a = atp.tile([128, 512], BF16, tag="a")
                            nc.vector.tensor_scalar_mul(a[:], e[:], rec[:])
                            # A^T chunks [(t',n)=128, (g,t)=128] via PE transpose.
                            at = atp.tile([128, 4, 128], BF16, tag="at")
                            for c in range(4):
                                tps = pstr.tile([128, 128], BF16, tag="tps")
                                nc.tensor.transpose(
                                    tps[:], a[:, c * 128 : (c + 1) * 128], ident[:]
                                )
                                nc.vector.tensor_copy(at[:, c, :], tps[:])
                            # attnout^T[d, (g,t)] = sum_c V_chunk.T @ A^T_chunk.
                            avps = psav.tile([128, 128], F32, tag="av")
                            for c in range(4):
                                nc.tensor.matmul(
                                    avps[:],
                                    vflat[:, kv, grp * 4 + c, :],
                                    at[:, c, :],
                                    start=(c == 0),
                                    stop=(c == 3),
                                )
                            nc.vector.tensor_copy(
                                aoT_sb[
                                    :,
                                    kv * G : (kv + 1) * G,
                                    t0 + grp * 32 : t0 + grp * 32 + 32,
                                ],
                                avps[:].rearrange("p (g t) -> p g t", g=G),
                            )

            # ---- Stage 3: out[t, hid] = attnout^T-chunks.T x WoT ----
            with (
                tc.tile_pool(name="wo", bufs=2) as wop,
                tc.tile_pool(name="osb", bufs=3) as osp,
                tc.tile_pool(name="ps3", bufs=2, space="PSUM") as ps3,
            ):
                for f in range(HID // 512):
                    wsl = wop.tile([128, H * D // 128, 512], BF16, tag="wo")
                    nc.sync.dma_start(wsl[:], wo_r[:, :, f * 512 : (f + 1) * 512])
                    for m in range(T // 128):
                        ops_ = ps3.tile([128, 512], F32, tag="ps3")
                        for k in range(H * D // 128):
                            nc.tensor.matmul(
                                ops_[:],
                                aoT_sb[:, k, m * 128 : (m + 1) * 128],
                                wsl[:, k, :],
                                start=(k == 0),
                                stop=(k == H * D // 128 - 1),
                            )
                        ob = osp.tile([128, 512], F32, tag="ob")
                        nc.scalar.copy(ob[:], ops_[:])
                        nc.sync.dma_start(out_r[:, m, f * 512 : (f + 1) * 512], ob[:])

    nc.compile()
    return nc


def _get_nc():
    global _NC
    if _NC is None:
        _NC = _build_program()
    return _NC


def _mask_neg() -> np.ndarray:
    rows = np.arange(128)[:, None]
    cols = np.arange(512)[None, :]
    return np.where(cols // N == rows % 32, 0.0, MASK_NEG).astype(np.float32)


def build_in_maps(hidden_states, retrieved_hidden_states, Wq, Wk, Wv, Wo):
    """Host-side sharding: pre-transpose + bf16-cast, slice tokens per core."""
    bf = ml_dtypes.bfloat16
    h2 = np.asarray(hidden_states, dtype=np.float32).reshape(TOK, HID).astype(bf)
    r2 = (
        np.asarray(retrieved_hidden_states, dtype=np.float32)
        .reshape(TOK * N, RH)
        .astype(bf)
    )
    wqT = np.ascontiguousarray(np.asarray(Wq, dtype=np.float32).astype(bf).T)
    wkT = np.ascontiguousarray(np.asarray(Wk, dtype=np.float32).astype(bf).T)
    wvT = np.ascontiguousarray(np.asarray(Wv, dtype=np.float32).astype(bf).T)
    woT = np.ascontiguousarray(np.asarray(Wo, dtype=np.float32).astype(bf).T)
    mneg = _mask_neg()

    in_maps = []
    for i in range(NCORES):
        hT_i = np.ascontiguousarray(h2[i * T : (i + 1) * T].T)
        rT_i = np.ascontiguousarray(r2[i * TN : (i + 1) * TN].T)
        in_maps.append(
            {
                "hT": hT_i,
                "rT": rT_i,
                "wqT": wqT,
                "wkT": wkT,
                "wvT": wvT,
                "woT": woT,
                "mneg": mneg,
            }
        )
    return in_maps


def kernel(hidden_states, retrieved_hidden_states, Wq, Wk, Wv, Wo):
    nc = _get_nc()
    in_maps = build_in_maps(
        hidden_states, retrieved_hidden_states, Wq, Wk, Wv, Wo
    )
    res = run_bass_kernel_spmd(nc, in_maps, core_ids=list(range(NCORES)))
    outs = [res.results[i]["out"] for i in range(NCORES)]
    full = np.concatenate(outs, axis=0).reshape(B, S, HID)
    return full



# revision 10
# speedup vs baseline: 76.8613x; 76.8613x over previous
"""Trainium2 Bass kernel for nn_CacheAttention (retrieval KNN attention).

Reference computation (per token, fully independent across tokens):
    q = (hidden @ Wq.T) * D**-0.5          # [t, H*D] viewed [t, KV, G, D]
    k = retrieved @ Wk.T                   # [t, N, KV*D] viewed [t, KV, N, D]
    v = retrieved @ Wv.T                   # viewed [t, KV, N, D]
    s = einsum('kgd,knd->kgn', q_t, k_t);  a = softmax(s, -1)
    out_t = einsum('kgn,knd->kgd', a, v_t).reshape(H*D) @ Wo.T

Strategy: data-parallel over the 4096 (b, s) tokens across 8 NeuronCores
(512 tokens each).  The host pre-transposes + bf16-casts all operands so
every matmul contracts over the SBUF partition dim with cheap, contiguous
DMA loads.  Attention uses a block-diagonal PE trick: scores for 32 tokens
x 4 groups in one PSUM tile [(g,t)=128, (t',n)=512], additive -30 mask +
exp (denominator via ACT accum), PE transpose of A, then a second
block-diag matmul against V for the weighted sum.

DMA schedule is tuned so the PE never waits on the serial HWDGE ring:
hT is loaded in chunks interleaved with the first Wq slabs (first matmul
issues ~2us in), the stage-2 operands (mask, Wk, Wv, first two retrieved
tiles) ride in mid-stage-1, and the first Wo slab prefetches in halves
mid-stage-2.
"""

import os
import sys

import numpy as np
import ml_dtypes

for _p in ("/opt/trn_rl_repo", "/root/.axon_site/_ro/trn_rl_repo"):
    if os.path.isdir(_p) and _p not in sys.path:
        sys.path.insert(0, _p)

import concourse.bass as bass  # noqa: E402
import concourse.mybir as mybir  # noqa: E402
import concourse.tile as tile  # noqa: E402
from concourse import bacc  # noqa: E402
from concourse.bass_utils import run_bass_kernel_spmd  # noqa: E402
from concourse.masks import make_identity  # noqa: E402

# Problem shapes (hardcoded per contest contract).
B, S, HID = 2, 2048, 4096
H, KV, D = 32, 8, 128
G = H // KV  # 4
N = 16
RH = HID // 4  # 1024
NCORES = 8
TOK = B * S  # 4096 tokens total
T = TOK // NCORES  # 512 tokens per core
TBLK = 32  # tokens per pipeline block
NBLK = T // TBLK  # 16
TN = T * N  # 8192 (token, neighbor) rows per core
TNBLK = TBLK * N  # 512
SCALE = float(D) ** -0.5
MASK_NEG = -30.0

BF16 = mybir.dt.bfloat16
F32 = mybir.dt.float32
EXP = mybir.ActivationFunctionType.Exp

_NC = None


def _stage1(nc, tc, ios, resp, qT_w):
    """Q^T[(kv,g,d), t] = WqT.T-chunks x hT, with interleaved DMA schedule.

    k-outer: each Wq slab is consumed by 4 back-to-back matmuls (one per
    128-col psum group) right after its 0.5us DMA lands, and hT arrives in
    8 chunks interleaved with the slabs -- the PE starts ~2us in instead of
    waiting out one 12.6us hT DMA.  Stage-2 operands (mask, Wk, Wv, first
    two retrieved tiles, first Wo half-slabs ride later) are deferred to
    mid-stage points so they never delay the slabs on the FIFO HWDGE ring.
    """
    (hT_r, rT_r, wq_r, wk_r, wv_r, _, mneg) = ios["dram"]
    mneg_sb, wk_sb, wv_sb, rt0, rt1 = ios["s2ops"]

    def _boundary_dma(ms, k):
        if (ms, k) == (1, 8):
            nc.sync.dma_start(mneg_sb[:], mneg[:])
        elif (ms, k) == (1, 24):
            nc.sync.dma_start(wk_sb[:, 0:4, :], wk_r[:, 0:4, :])
        elif (ms, k) == (2, 8):
            nc.sync.dma_start(wk_sb[:, 4:8, :], wk_r[:, 4:8, :])
        elif (ms, k) == (2, 24):
            nc.sync.dma_start(wv_sb[:, 0:4, :], wv_r[:, 0:4, :])
        elif (ms, k) == (3, 8):
            nc.sync.dma_start(wv_sb[:, 4:8, :], wv_r[:, 4:8, :])
        elif (ms, k) == (4, 8):
            nc.sync.dma_start(rt0[:], rT_r[:, :, 0:TNBLK])
        elif (ms, k) == (5, 8):
            nc.sync.dma_start(rt1[:], rT_r[:, :, TNBLK : 2 * TNBLK])

    with (
        tc.tile_pool(name="hpool", bufs=1) as hp,
        tc.tile_pool(name="wq", bufs=16) as wqp,
        tc.tile_pool(name="ps1", bufs=2, space="PSUM") as ps1,
    ):
        hT_sb = hp.tile([128, HID // 128, T], BF16)
        for ms in range(8):  # 512-col slabs of Wq^T
            qps = [
                ps1.tile([128, 512], F32, tag=f"qps{mi}", name=f"qps{mi}")
                for mi in range(4)
            ]
            for k in range(HID // 128):
                sl = wqp.tile([128, 512], BF16, tag="wqslab")
                nc.sync.dma_start(sl[:], wq_r[:, k, ms * 512 : (ms + 1) * 512])
                if ms == 0 and k % 4 == 0:
                    c = k // 4
                    nc.sync.dma_start(
                        hT_sb[:, c * 4 : (c + 1) * 4, :],
                        hT_r[:, c * 4 : (c + 1) * 4, :],
                    )
                _boundary_dma(ms, k)
                for mi in range(4):
                    nc.tensor.matmul(
                        qps[mi][:],
                        sl[:, mi * 128 : (mi + 1) * 128],
                        hT_sb[:, k, :],
                        start=(k == 0),
                        stop=(k == HID // 128 - 1),
                    )
            for mi in range(4):
                m = ms * 4 + mi
                # Fold the D**-0.5 query scaling into the PSUM evict.
                nc.scalar.mul(
                    qT_w[:, m // G, :, m % G, :],
                    qps[mi][:].rearrange("p (a t) -> p a t", t=32),
                    SCALE,
                )


def _stage2(nc, tc, ios, ident, qT_sb, aoT_sb):
    """Per 32-token block: K/V projections, gather-evictions, attention."""
    (hT_r, rT_r, wq_r, wk_r, wv_r, wo_r, mneg) = ios["dram"]
    mneg_sb, wk_sb, wv_sb, rt0, rt1 = ios["s2ops"]
    wsl0a, wsl0b = ios["wsl0"]
    rtiles = [rt0, rt1]

    with (
        tc.tile_pool(name="kt", bufs=2) as ktp,
        tc.tile_pool(name="vt", bufs=2) as vtp,
        tc.tile_pool(name="attn", bufs=2) as atp,
        tc.tile_pool(name="ps2", bufs=2, space="PSUM") as ps2,
        tc.tile_pool(name="pss", bufs=2, space="PSUM") as pss,
        tc.tile_pool(name="pstr", bufs=2, space="PSUM") as pstr,
        tc.tile_pool(name="psav", bufs=2, space="PSUM") as psav,
    ):
        for blk in range(NBLK):
            t0 = blk * TBLK
            rtile = rtiles[blk % 2]
            if blk >= 2:
                nc.sync.dma_start(
                    rtile[:], rT_r[:, :, blk * TNBLK : (blk + 1) * TNBLK]
                )
            if blk == 12:
                nc.sync.dma_start(wsl0a[:], wo_r[:, 0 : H * D // 256, 0:512])
            elif blk == 14:
                nc.sync.dma_start(
                    wsl0b[:], wo_r[:, H * D // 256 : H * D // 128, 0:512]
                )

            # The reference's torch flat-view [t,n,KV*D] -> [t,KV,n,D]
            # means head kv attends slot nn drawn from neighbor
            # n_src = 2*kv + nn//8 with kv-slice kvc = nn%8.  Softmax
            # is permutation-invariant per head, so we only need a
            # consistent slot order for K and V: slot = (n_src%2)*8
            # + kvc, gathered on the free dim during PSUM eviction.
            # K^T[d, head, (t, slot)] for this block.
            ktile = ktp.tile([128, KV, TNBLK], BF16, tag="kt")
            kdst = ktile[:].rearrange("p h (t b e) -> p t h b e", b=2, e=8)
            for kvc in range(KV):
                for f2 in range(TNBLK // 512):
                    kps = ps2.tile([128, 512], F32, tag="ps2")
                    for k in range(RH // 128):
                        nc.tensor.matmul(
                            kps[:],
                            wk_sb[:, k, kvc * 128 : (kvc + 1) * 128],
                            rtile[:, k, f2 * 512 : f2 * 512 + 512],
                            start=(k == 0),
                            stop=(k == RH // 128 - 1),
                        )
                    nc.scalar.copy(
                        kdst[:, f2 * 32 : (f2 + 1) * 32, :, :, kvc],
                        kps[:].rearrange("p (t a b) -> p t a b", a=8, b=2),
                    )

            # V^T[d, head, (t, slot)], same gather as K; then PE
            # transpose per head-chunk to V_flat[(t,slot), d].
            vht = vtp.tile([128, KV, TNBLK], BF16, tag="vht", bufs=1)
            vdst = vht[:].rearrange("p h (t b e) -> p t h b e", b=2, e=8)
            for kvc in range(KV):
                for f2 in range(TNBLK // 512):
                    vps = ps2.tile([128, 512], F32, tag="ps2")
                    for k in range(RH // 128):
                        nc.tensor.matmul(
                            vps[:],
                            wv_sb[:, k, kvc * 128 : (kvc + 1) * 128],
                            rtile[:, k, f2 * 512 : f2 * 512 + 512],
                            start=(k == 0),
                            stop=(k == RH // 128 - 1),
                        )
                    nc.scalar.copy(
                        vdst[:, f2 * 32 : (f2 + 1) * 32, :, :, kvc],
                        vps[:].rearrange("p (t a b) -> p t a b", a=8, b=2),
                    )
            vflat = vtp.tile([128, KV, TNBLK // 128, D], BF16, tag="vflat")
            for kv in range(KV):
                for c in range(TNBLK // 128):
                    vtp_ps = pstr.tile([128, 128], BF16, tag="tps")
                    nc.tensor.transpose(
                        vtp_ps[:], vht[:, kv, c * 128 : (c + 1) * 128], ident[:]
                    )
                    nc.vector.tensor_copy(vflat[:, kv, c, :], vtp_ps[:])

            # Attention for each (kv head, 32-token group).
            for kv in range(KV):
                for grp in range(TBLK // 32):
                    # Scores: psum [(g,t)=128, (t',n)=512]; only the
                    # block-diagonal (t'==t) 16-col slices are valid.
                    # The stationary packs all 4 GQA groups x 32
                    # tokens as one contiguous [128, 128] slice of
                    # the (grp, g, t) Q layout (walrus rejects >2D
                    # weight APs, so the layout bakes the packing).
                    sps = pss.tile([128, 512], F32, tag="sps")
                    gg = (t0 + grp * 32) // 32
                    nc.tensor.matmul(
                        sps[:],
                        qT_sb[:, kv, gg * 128 : (gg + 1) * 128],
                        ktile[:, kv, grp * 512 : grp * 512 + 512],
                        start=True,
                        stop=True,
                    )
                    sm = atp.tile([128, 512], F32, tag="sm")
                    nc.vector.tensor_add(sm[:], sps[:], mneg_sb[:])
                    e = atp.tile([128, 512], BF16, tag="e")
                    den = atp.tile([128, 1], F32, tag="den")
                    nc.scalar.activation(e[:], sm[:], EXP, accum_out=den[:])
                    rec = atp.tile([128, 1], F32, tag="rec")
                    nc.vector.reciprocal(rec[:], den[:])
                    a = atp.tile([128, 512], BF16, tag="a")
                    nc.vector.tensor_scalar_mul(a[:], e[:], rec[:])
                    # A^T chunks [(t',n)=128, (g,t)=128] via PE transpose.
                    at = atp.tile([128, 4, 128], BF16, tag="at")
                    for c in range(4):
                        tps = pstr.tile([128, 128], BF16, tag="tps")
                        nc.tensor.transpose(
                            tps[:], a[:, c * 128 : (c + 1) * 128], ident[:]
                        )
                        nc.vector.tensor_copy(at[:, c, :], tps[:])
                    # attnout^T[d, (g,t)] = sum_c V_chunk.T @ A^T_chunk.
                    avps = psav.tile([128, 128], F32, tag="av")
                    for c in range(4):
                        nc.tensor.matmul(
                            avps[:],
                            vflat[:, kv, grp * 4 + c, :],
                            at[:, c, :],
                            start=(c == 0),
                            stop=(c == 3),
                        )
                    nc.vector.tensor_copy(
                        aoT_sb[
                            :,
                            kv * G : (kv + 1) * G,
                            t0 + grp * 32 : t0 + grp * 32 + 32,
                        ],
                        avps[:].rearrange("p (g t) -> p g t", g=G),
                    )


def _stage3(nc, tc, ios, aoT_sb, out_r):
    """out[t, hid] = attnout^T-chunks.T x WoT."""
    wo_r = ios["dram"][5]
    wsl0a, wsl0b = ios["wsl0"]
    with (
        tc.tile_pool(name="wo", bufs=2) as wop,
        tc.tile_pool(name="osb", bufs=3) as osp,
        tc.tile_pool(name="ps3", bufs=2, space="PSUM") as ps3,
    ):
        KH = H * D // 256  # 16: k-chunks per prefetched half slab
        for f in range(HID // 512):
            if f == 0:
                halves = (wsl0a, wsl0b)
            else:
                wsl = wop.tile([128, H * D // 128, 512], BF16, tag="wo")
                nc.sync.dma_start(wsl[:], wo_r[:, :, f * 512 : (f + 1) * 512])
                halves = (wsl[:, 0:KH, :], wsl[:, KH : 2 * KH, :])
            for m in range(T // 128):
                ops_ = ps3.tile([128, 512], F32, tag="ps3")
                for k in range(H * D // 128):
                    nc.tensor.matmul(
                        ops_[:],
                        aoT_sb[:, k, m * 128 : (m + 1) * 128],
                        halves[k // KH][:, k % KH, :],
                        start=(k == 0),
                        stop=(k == H * D // 128 - 1),
                    )
                ob = osp.tile([128, 512], F32, tag="ob")
                nc.scalar.copy(ob[:], ops_[:])
                nc.sync.dma_start(out_r[:, m, f * 512 : (f + 1) * 512], ob[:])


def _build_program(reps=1):
    nc = bacc.Bacc(None, target_bir_lowering=False, debug=False)

    hT = nc.dram_tensor("hT", [HID, T], BF16, kind="ExternalInput")
    rT = nc.dram_tensor("rT", [RH, TN], BF16, kind="ExternalInput")
    wqT = nc.dram_tensor("wqT", [HID, H * D], BF16, kind="ExternalInput")
    wkT = nc.dram_tensor("wkT", [RH, KV * D], BF16, kind="ExternalInput")
    wvT = nc.dram_tensor("wvT", [RH, KV * D], BF16, kind="ExternalInput")
    woT = nc.dram_tensor("woT", [H * D, HID], BF16, kind="ExternalInput")
    mneg = nc.dram_tensor("mneg", [128, 512], F32, kind="ExternalInput")
    out = nc.dram_tensor("out", [T, HID], F32, kind="ExternalOutput")

    hT_r = hT[:].rearrange("(c p) t -> p c t", p=128)
    rT_r = rT[:].rearrange("(c p) t -> p c t", p=128)
    wq_r = wqT[:].rearrange("(c p) m -> p c m", p=128)
    wk_r = wkT[:].rearrange("(c p) m -> p c m", p=128)
    wv_r = wvT[:].rearrange("(c p) m -> p c m", p=128)
    wo_r = woT[:].rearrange("(c p) h -> p c h", p=128)
    out_r = out[:].rearrange("(mt p) h -> p mt h", p=128)

    with tile.TileContext(nc) as tc:
      for _rep in range(reps):  # >1 only for timing calibration builds
        with tc.tile_pool(name="resident", bufs=1) as resp:
            ident = resp.tile([128, 128], BF16)
            make_identity(nc, ident[:])
            mneg_sb = resp.tile([128, 512], F32)
            wk_sb = resp.tile([128, RH // 128, KV * D], BF16)
            wv_sb = resp.tile([128, RH // 128, KV * D], BF16)
            # Two retrieved tiles, manually double-buffered across the 16
            # stage-2 blocks; blocks 0/1 prefetch during stage 1.
            rt0 = resp.tile([128, RH // 128, TNBLK], BF16)
            rt1 = resp.tile([128, RH // 128, TNBLK], BF16)
            # First Wo slab (two k-halves), prefetched mid-stage-2 so
            # stage 3 doesn't stall 12.6us on its first weight load.
            wsl0a = resp.tile([128, H * D // 256, 512], BF16)
            wsl0b = resp.tile([128, H * D // 256, 512], BF16)
            aoT_sb = resp.tile([128, H * D // 128, T], BF16)

            ios = {
                "dram": (hT_r, rT_r, wq_r, wk_r, wv_r, wo_r, mneg),
                "s2ops": (mneg_sb, wk_sb, wv_sb, rt0, rt1),
                "wsl0": (wsl0a, wsl0b),
            }

            # Q^T lives in stages 1-2 only; scoping it frees 32KB/partition
            # for the stage-3 Wo double buffer.
            # Layout [d, kv, (grp, g, t32)]: the scores stationary for
            # (kv, grp) is a contiguous [128, 128] slice covering all 4 GQA
            # groups x 32 tokens.
            with tc.tile_pool(name="qtp", bufs=1) as qtp:
                qT_sb = qtp.tile([128, KV, G * T], BF16)
                qT_w = qT_sb[:].rearrange("p h (a g t) -> p h a g t", g=G, t=32)
                _stage1(nc, tc, ios, resp, qT_w)
                _stage2(nc, tc, ios, ident, qT_sb, aoT_sb)
            _stage3(nc, tc, ios, aoT_sb, out_r)

    nc.compile()
    return nc


def _get_nc():
    global _NC
    if _NC is None:
        _NC = _build_program()
    return _NC


def _mask_neg() -> np.ndarray:
    rows = np.arange(128)[:, None]
    cols = np.arange(512)[None, :]
    return np.where(cols // N == rows % 32, 0.0, MASK_NEG).astype(np.float32)


def build_in_maps(hidden_states, retrieved_hidden_states, Wq, Wk, Wv, Wo):
    """Host-side sharding: pre-transpose + bf16-cast, slice tokens per core."""
    bf = ml_dtypes.bfloat16
    h2 = np.asarray(hidden_states, dtype=np.float32).reshape(TOK, HID).astype(bf)
    r2 = (
        np.asarray(retrieved_hidden_states, dtype=np.float32)
        .reshape(TOK * N, RH)
        .astype(bf)
    )
    wqT = np.ascontiguousarray(np.asarray(Wq, dtype=np.float32).astype(bf).T)
    wkT = np.ascontiguousarray(np.asarray(Wk, dtype=np.float32).astype(bf).T)
    wvT = np.ascontiguousarray(np.asarray(Wv, dtype=np.float32).astype(bf).T)
    woT = np.ascontiguousarray(np.asarray(Wo, dtype=np.float32).astype(bf).T)
    mneg = _mask_neg()

    in_maps = []
    for i in range(NCORES):
        hT_i = np.ascontiguousarray(h2[i * T : (i + 1) * T].T)
        rT_i = np.ascontiguousarray(r2[i * TN : (i + 1) * TN].T)
        in_maps.append(
            {
                "hT": hT_i,
                "rT": rT_i,
                "wqT": wqT,
                "wkT": wkT,
                "wvT": wvT,
                "woT": woT,
                "mneg": mneg,
            }
        )
    return in_maps


def kernel(hidden_states, retrieved_hidden_states, Wq, Wk, Wv, Wo):
    nc = _get_nc()
    in_maps = build_in_maps(
        hidden_states, retrieved_hidden_states, Wq, Wk, Wv, Wo
    )
    res = run_bass_kernel_spmd(nc, in_maps, core_ids=list(range(NCORES)))
    outs = [res.results[i]["out"] for i in range(NCORES)]
    full = np.concatenate(outs, axis=0).reshape(B, S, HID)
    return full
